# revision 35
# baseline (speedup 1.0000x reference)
"""CharRNN Trainium2 kernel: 3-layer tanh RNN, B=512, T=1024, H=100, V=62.

Data-parallel over batch: 64 sequences per NeuronCore x 8 cores, weights
replicated, no collectives.

Per-core schedule ("ticks" of 8 time steps):
  - layer l processes time-block (tick - 2l); all three layers' recurrences
    run concurrently, one step apart, on a shared diagonal.
  - per step: one W_hh matmul per layer accumulates onto a PSUM bank that was
    pre-filled with the input projection (xin) by a big per-block GEMM; a
    single joint Tanh ACT instruction reads all active layers' PSUM slices
    (strided across banks) and writes the new hidden states into SBUF rings.
  - embedding lookup is a one-hot matmul: ids are broadcast across partitions
    with a K=1 matmul, compared against an iota vector on the vector engine,
    and the resulting one-hot hits a combined (emb @ W_ih0^T + bias) table.
  - decoder: y2 chunks (128 tokens) as the stationary operand against
    W_dec^T gives token-major [128, 62] logits; bias via the rings' ones-row.
"""

import numpy as np
from contextlib import ExitStack

import concourse.bass as bass
import concourse.tile as tile
from concourse import bacc, mybir
from concourse.bass_utils import run_bass_kernel_spmd

F32 = mybir.dt.float32
AF = mybir.ActivationFunctionType
ALU = mybir.AluOpType

VOCAB = 62
HID = 100
NL = 3
B, T = 512, 1024
NCORES = 8
BL = B // NCORES          # 64 sequences per core
SPB = 8                   # steps per block
WBLK = SPB * BL           # 512 columns per block


def trace_kernel(tc, outs, ins, nblk):
    nc = tc.nc
    logits, hidden = outs
    (ids_rows, e0p, wih1p, wih2p, whh0p, whh1p, whh2p, wdecp, onescol,
     iota62, ident, onesrow) = ins

    ctx = ExitStack()
    consts = ctx.enter_context(tc.tile_pool(name="consts", bufs=1))
    ypool = ctx.enter_context(tc.tile_pool(name="y", bufs=1))
    stagp = ctx.enter_context(tc.tile_pool(name="stag", bufs=1))
    ltp = ctx.enter_context(tc.tile_pool(name="lt", bufs=3))
    htp = ctx.enter_context(tc.tile_pool(name="ht", bufs=3))
    recp = ctx.enter_context(tc.tile_pool(name="rec", bufs=1, space="PSUM"))
    bcp = ctx.enter_context(tc.tile_pool(name="bc", bufs=1, space="PSUM"))
    decp_pool = ctx.enter_context(tc.tile_pool(name="dec", bufs=1, space="PSUM"))

    # ---- constants into SBUF
    def cload(ap, shape, nm):
        t = consts.tile(list(shape), F32, tag=nm, name=nm)
        nc.sync.dma_start(t[:], ap[:])
        return t

    e0_sb = cload(e0p, (128, HID), "e0_sb")
    wih_sb = [None, cload(wih1p, (128, HID), "wih1_sb"),
              cload(wih2p, (128, HID), "wih2_sb")]
    whh_sb = [cload(whh0p, (128, HID), "whh0_sb"),
              cload(whh1p, (128, HID), "whh1_sb"),
              cload(whh2p, (128, HID), "whh2_sb")]
    wdec_sb = cload(wdecp, (128, VOCAB), "wdec_sb")
    ones_sb = cload(onescol, (128, VOCAB), "ones_sb")
    iota_sb = cload(iota62, (128, 1), "iota_sb")
    id_sb = cload(ident, (128, 128), "id_sb")

    # ---- persistent state tiles
    # rings: [128, layer, slot, col]; rows 0..99 = h, row 100 = ones, rest 0
    Y = ypool.tile([128, 4, NL, WBLK], F32)   # [part, slot, layer, col]
    nc.vector.memset(Y[:], 0.0)
    nc.sync.dma_start(Y[100:101, :, :, :],
                      onesrow.rearrange("p (s l c) -> p s l c", s=4, l=NL))
    stag = [stagp.tile([128, WBLK], F32, tag=f"stag{i}", name=f"stag{i}")
            for i in range(2)]
    oh = [stagp.tile([128, WBLK], F32, tag=f"oh{i}", name=f"oh{i}")
          for i in range(2)]
    for tl in stag + oh:
        nc.vector.memset(tl[:], 0.0)

    # ---- PSUM: 3+3 recurrence banks (per block parity; separate tensors so
    # Tile's tensor-granular PSUM tracking doesn't serialize across parities),
    # 1 broadcast bank, 1 decoder bank
    recA = recp.tile([128, NL, WBLK], F32, tag="recA", name="recA")
    recB = recp.tile([128, NL, WBLK], F32, tag="recB", name="recB")
    recP = [recA, recB]
    bcps = bcp.tile([128, WBLK], F32)
    decps = decp_pool.tile([128, 128], F32)

    logr = logits.rearrange("b (blk q sl) v -> sl b blk q v", q=4, sl=2)

    PIECE = 256  # free-dim width of off-chain matmul pieces

    def emit_ids_dma(blk):
        nc.sync.dma_start(stag[blk % 2][0:1, :], ids_rows[blk:blk + 1, :])

    def emit_bc_piece(blk, p):
        c0 = p * PIECE
        nc.tensor.matmul(bcps[:VOCAB, c0:c0 + PIECE], ones_sb[:, :VOCAB],
                         stag[blk % 2][:, c0:c0 + PIECE],
                         start=(p == 0), stop=(c0 + PIECE == WBLK),
                         skip_group_check=True)

    def emit_iseq(blk):
        nc.vector.tensor_scalar(oh[blk % 2][:VOCAB, :], bcps[:VOCAB, :],
                                iota_sb[:VOCAB, :], None, ALU.is_equal)

    def emit_xin_piece(l, blk, p):
        c0 = p * PIECE
        out = recP[blk % 2][:HID, l, c0:c0 + PIECE]
        if l == 0:
            rhs = oh[blk % 2][:, c0:c0 + PIECE]
            lhsT = e0_sb[:, :HID]
        else:
            slot = (blk + 2 * (l - 1)) % 4
            rhs = Y[:, slot, l - 1, c0:c0 + PIECE]
            lhsT = wih_sb[l][:, :HID]
        nc.tensor.matmul(out, lhsT, rhs, start=(p == 0), stop=False,
                         skip_group_check=True)

    def emit_dec_chunk(D, q, lt):
        slot = (D + 4) % 4
        nc.tensor.matmul(decps[:, :VOCAB],
                         Y[:, slot, 2, q * 128:(q + 1) * 128],
                         wdec_sb[:, :VOCAB], start=True, stop=True)
        nc.vector.tensor_copy(lt[:, q, :], decps[:, :VOCAB])

    def emit_logit_dma(D, lt):
        for sl in range(2):
            nc.sync.dma_start(logr[sl, :, D, :, :], lt[sl * 64:(sl + 1) * 64])

    NP = WBLK // PIECE

    def emit_onehot(blk):
        emit_ids_dma(blk)
        for p in range(NP):
            emit_bc_piece(blk, p)
        emit_iseq(blk)

    def emit_xin(l, blk):
        for p in range(NP):
            emit_xin_piece(l, blk, p)

    def emit_decode(D):
        lt = ltp.tile([128, 4, VOCAB], F32)
        for q in range(4):
            emit_dec_chunk(D, q, lt)
        emit_logit_dma(D, lt)

    def emit_hidden(l, g):
        """Extract h(T-1) for layer l (tick g is its last)."""
        slot = g % 4
        nc.tensor.transpose(decps[:BL, :], Y[:, slot, l, 7 * BL:8 * BL], id_sb[:])
        ht = htp.tile([BL, HID], F32)
        nc.vector.tensor_copy(ht[:], decps[:BL, :HID])
        nc.sync.dma_start(hidden[l], ht[:])

    # ---- prologue: one-hot for blocks 0, 1; xin0 for block 0
    emit_onehot(0)
    if nblk > 1:
        emit_onehot(1)
    emit_xin(0, 0)

    # ---- main tick loop
    for g in range(nblk + 2 * (NL - 1) + 1):
        e = g % 2
        slot = g % 4
        active = [l for l in range(NL) if 0 <= g - 2 * l < nblk]

        # off-chain work for this tick, split into small pieces and
        # interleaved into the 8 step windows (<=2 per window) so each
        # piece hides in an ACT shadow on the in-order PE stream
        window_ops = []
        if g + 2 < nblk:
            window_ops.append(lambda blk=g + 2: emit_ids_dma(blk))
            for p in range(NP):
                window_ops.append(lambda blk=g + 2, p=p: emit_bc_piece(blk, p))
            window_ops.append(lambda blk=g + 2: emit_iseq(blk))
        if g + 1 < nblk:
            for p in range(NP):
                window_ops.append(
                    lambda blk=g + 1, p=p: emit_xin_piece(0, blk, p))
        if 0 <= g - 1 < nblk:
            for p in range(NP):
                window_ops.append(
                    lambda blk=g - 1, p=p: emit_xin_piece(1, blk, p))
        if 0 <= g - 3 < nblk:
            for p in range(NP):
                window_ops.append(
                    lambda blk=g - 3, p=p: emit_xin_piece(2, blk, p))
        if 0 <= g - 5 < nblk:
            lt = ltp.tile([128, 4, VOCAB], F32)
            for q in range(4):
                window_ops.append(
                    lambda D=g - 5, q=q, lt=lt: emit_dec_chunk(D, q, lt))
            window_ops.append(lambda D=g - 5, lt=lt: emit_logit_dma(D, lt))

        for s in range(SPB):
            for l in active:
                blk = g - 2 * l
                t_step = blk * SPB + s
                if t_step == 0:
                    continue  # h_{-1} = 0: PSUM already holds xin only
                if s > 0:
                    prev = Y[:, slot, l, (s - 1) * BL:s * BL]
                else:
                    prev = Y[:, (g - 1) % 4, l, 7 * BL:8 * BL]
                nc.tensor.matmul(recP[e][:HID, l, s * BL:(s + 1) * BL],
                                 whh_sb[l][:, :HID], prev,
                                 start=False, stop=(s == SPB - 1),
                                 skip_group_check=True)
            if active:
                lmin, lmax = active[0], active[-1]
                src = recP[e][:HID, lmin:lmax + 1, s * BL:(s + 1) * BL]
                dst = Y[:HID, slot, lmin:lmax + 1, s * BL:(s + 1) * BL]
                nc.scalar.activation(dst, src, AF.Tanh)
            for _ in range(2):
                if window_ops:
                    window_ops.pop(0)()

        for op in window_ops:  # any leftovers (head/tail ticks)
            op()

        for l in range(NL):
            if g - 2 * l == nblk - 1:
                emit_hidden(l, g)

    ctx.close()


_CACHE = {}
TRACE = False
LAST_RESULT = None


def build(nblk=T // SPB):
    if nblk in _CACHE:
        return _CACHE[nblk]
    nc = bacc.Bacc("TRN2", target_bir_lowering=False, debug=False,
                   num_devices=NCORES)
    Tn = nblk * SPB
    ins = [
        nc.dram_tensor("ids_rows", [nblk, WBLK], F32, kind="ExternalInput").ap(),
        nc.dram_tensor("e0p", [128, HID], F32, kind="ExternalInput").ap(),
        nc.dram_tensor("wih1p", [128, HID], F32, kind="ExternalInput").ap(),
        nc.dram_tensor("wih2p", [128, HID], F32, kind="ExternalInput").ap(),
        nc.dram_tensor("whh0p", [128, HID], F32, kind="ExternalInput").ap(),
        nc.dram_tensor("whh1p", [128, HID], F32, kind="ExternalInput").ap(),
        nc.dram_tensor("whh2p", [128, HID], F32, kind="ExternalInput").ap(),
        nc.dram_tensor("wdecp", [128, VOCAB], F32, kind="ExternalInput").ap(),
        nc.dram_tensor("onescol", [128, VOCAB], F32, kind="ExternalInput").ap(),
        nc.dram_tensor("iota62", [128, 1], F32, kind="ExternalInput").ap(),
        nc.dram_tensor("ident", [128, 128], F32, kind="ExternalInput").ap(),
        nc.dram_tensor("onesrow", [1, NL * 4 * WBLK], F32,
                       kind="ExternalInput").ap(),
    ]
    outs = [
        nc.dram_tensor("logits_l", [BL, Tn, VOCAB], F32,
                       kind="ExternalOutput").ap(),
        nc.dram_tensor("hidden_l", [NL, BL, HID], F32,
                       kind="ExternalOutput").ap(),
    ]
    with tile.TileContext(nc) as tc:
        trace_kernel(tc, outs, ins, nblk)
    nc.compile()
    _CACHE[nblk] = nc
    return nc


def host_prep(emb, W_ih, W_hh, b_ih, b_hh, W_dec, b_dec):
    def pad(rows, arr):
        out = np.zeros((128, arr.shape[1]), dtype=np.float32)
        out[:rows] = arr
        return out

    bias = (b_ih + b_hh).astype(np.float32)          # [NL, HID]
    e0 = emb.astype(np.float32) @ W_ih[0].astype(np.float32).T + bias[0]
    shared = {
        "e0p": pad(VOCAB, e0),
        "wih1p": pad(HID + 1, np.vstack([W_ih[1].T, bias[1][None]])),
        "wih2p": pad(HID + 1, np.vstack([W_ih[2].T, bias[2][None]])),
        "whh0p": pad(HID, W_hh[0].T),
        "whh1p": pad(HID, W_hh[1].T),
        "whh2p": pad(HID, W_hh[2].T),
        "wdecp": pad(HID + 1, np.vstack([W_dec.T.astype(np.float32),
                                         b_dec[None].astype(np.float32)])),
        "onescol": pad(1, np.ones((1, VOCAB), np.float32)),
        "iota62": pad(VOCAB, np.arange(VOCAB, dtype=np.float32)[:, None]),
        "ident": np.eye(128, dtype=np.float32),
        "onesrow": np.ones((1, NL * 4 * WBLK), np.float32),
    }
    shared = {k: np.ascontiguousarray(v.astype(np.float32))
              for k, v in shared.items()}
    return shared


def ids_to_rows(ids_local, nblk=T // SPB):
    # [BL, Tn] -> [nblk, WBLK] with col = s*BL + b
    Tn = nblk * SPB
    r = ids_local[:, :Tn].reshape(BL, nblk, SPB).transpose(1, 2, 0)
    return np.ascontiguousarray(r.reshape(nblk, WBLK).astype(np.float32))


def kernel(input_ids, emb, W_ih, W_hh, b_ih, b_hh, W_dec, b_dec):
    input_ids = np.asarray(input_ids)
    emb = np.asarray(emb, np.float32)
    W_ih = np.asarray(W_ih, np.float32)
    W_hh = np.asarray(W_hh, np.float32)
    b_ih = np.asarray(b_ih, np.float32)
    b_hh = np.asarray(b_hh, np.float32)
    W_dec = np.asarray(W_dec, np.float32)
    b_dec = np.asarray(b_dec, np.float32)

    nc = build()
    shared = host_prep(emb, W_ih, W_hh, b_ih, b_hh, W_dec, b_dec)
    in_maps = []
    for c in range(NCORES):
        m = dict(shared)
        m["ids_rows"] = ids_to_rows(input_ids[c * BL:(c + 1) * BL])
        in_maps.append(m)

    global LAST_RESULT
    res = run_bass_kernel_spmd(nc, in_maps, core_ids=list(range(NCORES)),
                               trace=TRACE)
    LAST_RESULT = res
    logits = np.concatenate([r["logits_l"] for r in res.results], axis=0)
    hid = np.concatenate([r["hidden_l"] for r in res.results], axis=1)
    return logits, hid


# revision 36
# speedup vs baseline: 1.2589x; 1.2589x over previous
"""CharRNN Trainium2 kernel: 3-layer tanh RNN, B=512, T=1024, H=100, V=62.

Data-parallel over batch: 64 sequences per NeuronCore x 8 cores, weights
replicated, no collectives.

Per-core schedule ("ticks" of 8 time steps):
  - layer l processes time-block (tick - 2l); all three layers' recurrences
    run concurrently, one step apart, on a shared diagonal.
  - per step: one W_hh matmul per layer accumulates onto a PSUM bank that was
    pre-filled with the input projection (xin) by a big per-block GEMM; a
    single joint Tanh ACT instruction reads all active layers' PSUM slices
    (strided across banks) and writes the new hidden states into SBUF rings.
  - embedding lookup is a one-hot matmul: ids are broadcast across partitions
    with a K=1 matmul, compared against an iota vector on the vector engine,
    and the resulting one-hot hits a combined (emb @ W_ih0^T + bias) table.
  - decoder: y2 chunks (128 tokens) as the stationary operand against
    W_dec^T gives token-major [128, 62] logits; bias via the rings' ones-row.
"""

import numpy as np
from contextlib import ExitStack

import concourse.bass as bass
import concourse.tile as tile
from concourse import bacc, mybir
from bass_rust import add_dep_helper
from concourse.bass_utils import run_bass_kernel_spmd

F32 = mybir.dt.float32
AF = mybir.ActivationFunctionType
ALU = mybir.AluOpType

VOCAB = 62
HID = 100
NL = 3
B, T = 512, 1024
NCORES = 8
BL = B // NCORES          # 64 sequences per core
SPB = 4                   # steps per block
WBLK = SPB * BL           # 512 columns per block


def trace_kernel(tc, outs, ins, nblk):
    nc = tc.nc
    logits, hidden = outs
    (oh_rows, e0p, wih1p, wih2p, whh0p, whh1p, whh2p, wdecp, onescol,
     iota62, ident, onesrow) = ins

    ctx = ExitStack()
    consts = ctx.enter_context(tc.tile_pool(name="consts", bufs=1))
    ypool = ctx.enter_context(tc.tile_pool(name="y", bufs=1))
    stagp = ctx.enter_context(tc.tile_pool(name="stag", bufs=1))
    ltp = ctx.enter_context(tc.tile_pool(name="lt", bufs=3))
    htp = ctx.enter_context(tc.tile_pool(name="ht", bufs=3))
    recp = ctx.enter_context(tc.tile_pool(name="rec", bufs=1, space="PSUM"))
    bcp = ctx.enter_context(tc.tile_pool(name="bc", bufs=1, space="PSUM"))
    decp_pool = ctx.enter_context(tc.tile_pool(name="dec", bufs=1, space="PSUM"))

    # ---- constants into SBUF
    def cload(ap, shape, nm):
        t = consts.tile(list(shape), F32, tag=nm, name=nm)
        nc.sync.dma_start(t[:], ap[:])
        return t

    e0_sb = cload(e0p, (128, HID), "e0_sb")
    wih_sb = [None, cload(wih1p, (128, HID), "wih1_sb"),
              cload(wih2p, (128, HID), "wih2_sb")]
    whh_sb = [cload(whh0p, (128, HID), "whh0_sb"),
              cload(whh1p, (128, HID), "whh1_sb"),
              cload(whh2p, (128, HID), "whh2_sb")]
    wdec_sb = cload(wdecp, (128, VOCAB), "wdec_sb")
    ones_sb = cload(onescol, (128, VOCAB), "ones_sb")
    iota_sb = cload(iota62, (128, 1), "iota_sb")
    id_sb = cload(ident, (128, 128), "id_sb")

    # ---- persistent state tiles
    # rings: [part, slot, half, step, layer*32+b]; rows 0..99 = h,
    # row 100 = ones, rest 0. Layer-adjacent columns make the joint tanh
    # a plain 2D contiguous access on both PSUM and SBUF sides.
    HB = BL // 2
    SH = SPB * HB            # cols per (half, layer)
    Y = ypool.tile([128, 4, 2, NL, SH], F32)
    nc.vector.memset(Y[:], 0.0)
    nc.sync.dma_start(Y[100:101],
                      onesrow.rearrange("p (a h l c) -> p a h l c",
                                        a=4, h=2, l=NL))
    oh = [stagp.tile([128, WBLK], F32, tag=f"oh{i}", name=f"oh{i}")
          for i in range(4)]
    for tl in oh:
        nc.vector.memset(tl[:], 0.0)

    # ---- PSUM: 3+3 recurrence banks (per block parity; separate tensors so
    # Tile's tensor-granular PSUM tracking doesn't serialize across parities),
    # 1 broadcast bank, 1 decoder bank. Layout [half, step, layer*32+b].
    recA0 = recp.tile([128, NL, SH], F32, tag="recA0", name="recA0")
    recA1 = recp.tile([128, NL, SH], F32, tag="recA1", name="recA1")
    recB0 = recp.tile([128, NL, SH], F32, tag="recB0", name="recB0")
    recB1 = recp.tile([128, NL, SH], F32, tag="recB1", name="recB1")
    recP = [[recA0, recA1], [recB0, recB1]]
    decps = decp_pool.tile([128, 128], F32, tag="decps", name="decps")
    decps2 = decp_pool.tile([128, 128], F32, tag="decps2", name="decps2")
    decD = [decps, decps2]

    logr = logits.rearrange("(hh bb) (blk sl) v -> hh blk sl bb v",
                            hh=2, sl=SPB)

    PIECE = 256  # free-dim width of off-chain matmul pieces

    def emit_oh_dma(blk):
        nc.sync.dma_start(oh[blk % 4][:VOCAB, :], oh_rows[blk])

    # xin pieces: one per (layer, half) — 2D contiguous cols within the
    # (parity, half) tensor's single bank. The first piece per tick into
    # each tensor carries start=True (clears has_written bank-wide; stale
    # columns of absent layers are never read).
    def emit_xin_piece(l, blk, h, start):
        out = recP[blk % 2][h][:HID, l, :]
        if l == 0:
            rhs = oh[blk % 4][:, h * SH:(h + 1) * SH]
        else:
            slot = (blk + 2 * (l - 1)) % 4
            rhs = Y[:, slot, h, l - 1, :]
        lhsT = e0_sb[:, :HID] if l == 0 else wih_sb[l][:, :HID]
        return nc.tensor.matmul(out, lhsT, rhs, start=start, stop=False,
                                skip_group_check=True)

    def emit_dec_chunk(D, q, lt):
        slot = (D + 4) % 4
        mm = nc.tensor.matmul(decD[q][:, :VOCAB],
                              Y[:, slot, q, NL - 1, :],
                              wdec_sb[:, :VOCAB], start=True, stop=True)
        nc.vector.tensor_copy(lt[:, q, :], decD[q][:, :VOCAB])
        return mm

    def emit_logit_dma(D, lt):
        for q in range(2):
            nc.sync.dma_start(logr[q, D], lt[:, q, :])

    NP = WBLK // PIECE

    def emit_xin(l, blk):
        for h in range(2):
            emit_xin_piece(l, blk, h, start=True)

    def emit_decode(D):
        lt = ltp.tile([128, 2, VOCAB], F32)
        for q in range(2):
            emit_dec_chunk(D, q, lt)
        emit_logit_dma(D, lt)

    def emit_hidden(l, g):
        """Extract h(T-1) for layer l (tick g is its last)."""
        slot = g % 4
        for h in range(2):
            nc.tensor.transpose(decD[h][:HB, :],
                                Y[:, slot, h, l, (SPB - 1) * HB:SPB * HB],
                                id_sb[:])
        ht = htp.tile([BL, HID], F32)
        for h in range(2):
            nc.vector.tensor_copy(ht[h * HB:(h + 1) * HB], decD[h][:HB, :HID])
        nc.sync.dma_start(hidden[l], ht[:])

    # ---- prologue: host-encoded one-hot for blocks 0-3, xin0 for block 0
    for b0 in range(min(4, nblk)):
        emit_oh_dma(b0)
    emit_xin(0, 0)

    # ---- main tick loop
    for g in range(nblk + 2 * (NL - 1) + 1):
        e = g % 2
        slot = g % 4
        active = [l for l in range(NL) if 0 <= g - 2 * l < nblk]

        # off-chain work for this tick: exactly one group per step window
        # (8 windows), each with at most ~2 small PE ops so nothing blocks
        # the in-order PE stream for long
        window_ops = []
        if g + 4 < nblk:
            emit_oh_dma(g + 4)  # DMA only; deep lookahead, no PE window
        seen_h = set()
        for l, boff in ((0, g + 1), (1, g - 1), (2, g - 3)):
            if 0 <= boff < nblk:
                for h in range(2):
                    st = h not in seen_h
                    seen_h.add(h)
                    window_ops.append(
                        lambda l=l, blk=boff, h=h, st=st:
                        emit_xin_piece(l, blk, h, st))
        if 0 <= g - 5 < nblk:
            def grp_dec(D=g - 5):
                lt = ltp.tile([128, 2, VOCAB], F32)
                mm = None
                for q in range(2):
                    mm = emit_dec_chunk(D, q, lt)
                emit_logit_dma(D, lt)
                return mm
            window_ops.append(grp_dec)


        pin = None  # PE inst of last window op; ordered before next round
        for s in range(SPB):
            for h in range(2):
                # reversed layer order on the second half reuses the last
                # stationary W_hh (fewer LDWEIGHTS switches)
                for l in (active if h == 0 else reversed(active)):
                    blk = g - 2 * l
                    t_step = blk * SPB + s
                    if t_step == 0:
                        continue  # h_{-1} = 0: PSUM already holds xin only
                    if s > 0:
                        prev = Y[:, slot, h, l, (s - 1) * HB:s * HB]
                    else:
                        prev = Y[:, (g - 1) % 4, h, l,
                                 (SPB - 1) * HB:SPB * HB]
                    mm = nc.tensor.matmul(
                        recP[e][h][:HID, l, s * HB:(s + 1) * HB],
                        whh_sb[l][:, :HID], prev,
                        start=False,
                        stop=(s == SPB - 1),
                        skip_group_check=True)
                    if pin is not None:
                        add_dep_helper(mm.ins, pin.ins, sync=False,
                                       reason="pin window op before round")
                        pin = None
                if active:
                    lmin, lmax = active[0], active[-1]
                    src = recP[e][h][:HID, lmin:lmax + 1,
                                     s * HB:(s + 1) * HB]
                    dst = Y[:HID, slot, h, lmin:lmax + 1,
                            s * HB:(s + 1) * HB]
                    nc.scalar.activation(dst, src, AF.Tanh)
                if window_ops:
                    pin = window_ops.pop(0)() or pin

        for op in window_ops:  # any leftovers (head/tail ticks)
            op()

        for l in range(NL):
            if g - 2 * l == nblk - 1:
                emit_hidden(l, g)

    ctx.close()


_CACHE = {}
TRACE = False
LAST_RESULT = None


def build(nblk=T // SPB):
    if nblk in _CACHE:
        return _CACHE[nblk]
    nc = bacc.Bacc("TRN2", target_bir_lowering=False, debug=False,
                   num_devices=NCORES)
    Tn = nblk * SPB
    ins = [
        nc.dram_tensor("oh_rows", [nblk, VOCAB, WBLK], F32,
                       kind="ExternalInput").ap(),
        nc.dram_tensor("e0p", [128, HID], F32, kind="ExternalInput").ap(),
        nc.dram_tensor("wih1p", [128, HID], F32, kind="ExternalInput").ap(),
        nc.dram_tensor("wih2p", [128, HID], F32, kind="ExternalInput").ap(),
        nc.dram_tensor("whh0p", [128, HID], F32, kind="ExternalInput").ap(),
        nc.dram_tensor("whh1p", [128, HID], F32, kind="ExternalInput").ap(),
        nc.dram_tensor("whh2p", [128, HID], F32, kind="ExternalInput").ap(),
        nc.dram_tensor("wdecp", [128, VOCAB], F32, kind="ExternalInput").ap(),
        nc.dram_tensor("onescol", [128, VOCAB], F32, kind="ExternalInput").ap(),
        nc.dram_tensor("iota62", [128, 1], F32, kind="ExternalInput").ap(),
        nc.dram_tensor("ident", [128, 128], F32, kind="ExternalInput").ap(),
        nc.dram_tensor("onesrow", [1, NL * 4 * WBLK], F32,
                       kind="ExternalInput").ap(),
    ]
    outs = [
        nc.dram_tensor("logits_l", [BL, Tn, VOCAB], F32,
                       kind="ExternalOutput").ap(),
        nc.dram_tensor("hidden_l", [NL, BL, HID], F32,
                       kind="ExternalOutput").ap(),
    ]
    with tile.TileContext(nc) as tc:
        trace_kernel(tc, outs, ins, nblk)
    nc.compile()
    _CACHE[nblk] = nc
    return nc


def host_prep(emb, W_ih, W_hh, b_ih, b_hh, W_dec, b_dec):
    def pad(rows, arr):
        out = np.zeros((128, arr.shape[1]), dtype=np.float32)
        out[:rows] = arr
        return out

    bias = (b_ih + b_hh).astype(np.float32)          # [NL, HID]
    e0 = emb.astype(np.float32) @ W_ih[0].astype(np.float32).T + bias[0]
    shared = {
        "e0p": pad(VOCAB, e0),
        "wih1p": pad(HID + 1, np.vstack([W_ih[1].T, bias[1][None]])),
        "wih2p": pad(HID + 1, np.vstack([W_ih[2].T, bias[2][None]])),
        "whh0p": pad(HID, W_hh[0].T),
        "whh1p": pad(HID, W_hh[1].T),
        "whh2p": pad(HID, W_hh[2].T),
        "wdecp": pad(HID + 1, np.vstack([W_dec.T.astype(np.float32),
                                         b_dec[None].astype(np.float32)])),
        "onescol": pad(1, np.ones((1, VOCAB), np.float32)),
        "iota62": pad(VOCAB, np.arange(VOCAB, dtype=np.float32)[:, None]),
        "ident": np.eye(128, dtype=np.float32),
        "onesrow": np.ones((1, NL * 4 * WBLK), np.float32),
    }
    shared = {k: np.ascontiguousarray(v.astype(np.float32))
              for k, v in shared.items()}
    return shared


def ids_to_rows(ids_local, nblk=T // SPB):
    # [BL, Tn] -> one-hot [nblk, VOCAB, WBLK], col = h*(SPB*32) + s*32 + b_lo
    Tn = nblk * SPB
    r = ids_local[:, :Tn].reshape(2, BL // 2, nblk, SPB)
    r = r.transpose(2, 0, 3, 1).reshape(nblk, WBLK)  # [blk, col]
    onehot = (r[:, None, :] == np.arange(VOCAB)[None, :, None])
    return np.ascontiguousarray(onehot.astype(np.float32))


def kernel(input_ids, emb, W_ih, W_hh, b_ih, b_hh, W_dec, b_dec):
    input_ids = np.asarray(input_ids)
    emb = np.asarray(emb, np.float32)
    W_ih = np.asarray(W_ih, np.float32)
    W_hh = np.asarray(W_hh, np.float32)
    b_ih = np.asarray(b_ih, np.float32)
    b_hh = np.asarray(b_hh, np.float32)
    W_dec = np.asarray(W_dec, np.float32)
    b_dec = np.asarray(b_dec, np.float32)

    nc = build()
    shared = host_prep(emb, W_ih, W_hh, b_ih, b_hh, W_dec, b_dec)
    in_maps = []
    for c in range(NCORES):
        m = dict(shared)
        m["oh_rows"] = ids_to_rows(input_ids[c * BL:(c + 1) * BL])
        in_maps.append(m)

    global LAST_RESULT
    res = run_bass_kernel_spmd(nc, in_maps, core_ids=list(range(NCORES)),
                               trace=TRACE)
    LAST_RESULT = res
    logits = np.concatenate([r["logits_l"] for r in res.results], axis=0)
    hid = np.concatenate([r["hidden_l"] for r in res.results], axis=1)
    return logits, hid
